# revision 29
# baseline (speedup 1.0000x reference)
"""Single-head causal attention on 8 Trainium2 NeuronCores (Bass/Tile).

Problem: x [4, 4096, 1024] f32, Wq/Wk/Wv [64, 1024] f32 ->
         softmax(causal(q k^T * H^-0.5)) v   -> [4, 4096, 64] f32

Sharding: core = (batch b, parity p), b = core//2, p = core%2. Each core owns
the global 128-wide query tiles g = 2j+p (j=0..15) of its batch -- the parity
interleave balances causal work AND keeps the compiled graph identical across
all 8 cores (SPMD: one NEFF). All parity differences live in host-prepared
data (shifted xt + dbias), never in the graph.

v3 design notes (v1 -> v2 -> v3 driven by perfetto traces):

  * xt/weights are PRE-SWIZZLED ON HOST so every DMA line is one
    per-partition-contiguous run: xt [128, 8 waves, 8 c, 512] -> a wave is
    128 descriptors of 8KB instead of 1024 of 1KB.  Weights ride the scalar
    ring, the 8 xt waves ride the sync ring, in parallel.
  * K^T stays at SBUF partitions 64:128 (where the fused [V^T;K^T]
    projection puts it); rows 0:64 of kt_sb are zeroed once and the
    duplicated Q^T rows make the 128-contraction correct.
  * exp runs on PAIRS of k-tiles: one ACT over a 2-bank PSUM tile
    [128, 2, 512] halves ScalarE's per-instruction init overhead.
  * AV is q-major: stationary = exp(S^T) subtile [128k, 128q], moving =
    V tile [128k, 65] -> out [128q, 65], 65-column streams whose weight
    loads hide behind the previous matmul (FWL + background buffer).  The
    output lands q-major: the epilogue is DVE-only (den/recip/scale per
    subtile, drip-fed at each diagonal pair) and ONE y DMA per chunk.
  * PSUM start_tensor_calc=True resets the WHOLE bank's has_written bits,
    so only the chunk's first AV write sets it; the other q-subtiles'
    first writes land on clear bits (which behave as overwrite).
  * The projection feed is re-phased so chunk 3 (no successor) still has
    KV7+V7 work to keep the PE p-state warm, and KV units sit late enough
    that the xt waves (2.9us apart on the wire) have landed.
"""
import os

import numpy as np
import ml_dtypes

import concourse.bass as bass
import concourse.mybir as mybir
import concourse.tile as tile
from concourse import bacc
from concourse.bass_utils import run_bass_kernel_spmd
from concourse.masks import make_identity

P = 128
B, T, C, H = 4, 4096, 1024, 64
TQ = T // 2          # queries per core
CH = 512             # q-chunk width
NCH = TQ // CH       # 4 q-chunks
CT = C // P          # 8 contraction tiles
NKT = T // P         # 32 k-tiles
NW = T // CH         # 8 xt DMA waves / projection chunks
N_CORES = 8
DEPTH = 2            # score PAIRS in flight (4 k-tiles)

F32 = mybir.dt.float32
BF16 = mybir.dt.bfloat16
Exp = mybir.ActivationFunctionType.Exp
MULT = mybir.AluOpType.mult
SUB = mybir.AluOpType.subtract

LAST_EXEC_TIME_NS = None
_COMPILED = None


def _build_graph():
    nc = bacc.Bacc("TRN2", target_bir_lowering=False, debug=False,
                   num_devices=N_CORES)
    xt = nc.dram_tensor("xt", [P, NW, CT, CH], BF16, kind="ExternalInput").ap()
    wqq = nc.dram_tensor("wqq", [P, CT, P], BF16, kind="ExternalInput").ap()
    wkv = nc.dram_tensor("wkv", [P, CT, P], BF16, kind="ExternalInput").ap()
    mtri = nc.dram_tensor("mtri", [P, P], BF16, kind="ExternalInput").ap()
    iden = nc.dram_tensor("iden", [P, P], BF16, kind="ExternalInput").ap()
    dbias = nc.dram_tensor("dbias", [P, 1], F32, kind="ExternalInput").ap()
    y = nc.dram_tensor("y", [TQ, H], F32, kind="ExternalOutput").ap()

    y_r = y.rearrange("(c s p) h -> c p s h", p=P, s=4)  # [4, 128, 4, 64]

    with tile.TileContext(nc) as tc:
        with (
            tc.tile_pool(name="const", bufs=1) as const,
            tc.tile_pool(name="ssb", bufs=6) as sspool,
            tc.tile_pool(name="epi", bufs=2) as epool,
            tc.tile_pool(name="pproj", bufs=1, space="PSUM") as ppool,
            tc.tile_pool(name="ps", bufs=2, space="PSUM") as spool,
            tc.tile_pool(name="po", bufs=2, space="PSUM") as opool,
            tc.tile_pool(name="pt", bufs=1, space="PSUM") as tpool,
        ):
            # ---- constants ----
            wqq_sb = const.tile([P, CT, P], BF16, name="wqq_sb")
            wkv_sb = const.tile([P, CT, P], BF16, name="wkv_sb")
            mask_sb = const.tile([P, P], BF16, name="mask_sb")
            dbias_sb = const.tile([P, 1], F32, name="dbias_sb")
            ident16 = const.tile([P, P], BF16, name="ident16")
            scratch = const.tile([P, 1], F32, name="scratch")
            # weights on the gpsimd ring (own SWDGE): their descriptors
            # round-robin with the sync-ring xt waves at the engines, so
            # both progress from ~8us; sync-ring FIFO or the starved scalar
            # ring would gate the first projection ~3-5us later
            nc.gpsimd.dma_start(wkv_sb[:], wkv)
            nc.gpsimd.dma_start(wqq_sb[:], wqq)
            nc.gpsimd.dma_start(mask_sb[:], mtri)
            nc.gpsimd.dma_start(ident16[:], iden)
            nc.gpsimd.dma_start(dbias_sb[:], dbias)
            # preload the exp table set immediately (scratch <- exp(0))
            nc.vector.memset(scratch[:], 0.0)
            nc.scalar.activation(scratch[:], scratch[:], Exp)

            # ---- resident x ----
            xt_sb = const.tile([P, NW, CT, CH], BF16, name="xt_sb")
            # odd 128-blocks hold this core's query tokens
            xt_qv = xt_sb.rearrange("p w co (bb two q) -> p w co bb two q",
                                    bb=2, two=2, q=P)

            # ---- persistent activations ----
            qt_sb = const.tile([P, TQ], BF16, name="qt_sb")      # Q^T dup rows
            kvt_sb = const.tile([P, T], BF16, name="kvt_sb")     # V^T | K^T
            kt_sb = const.tile([P, T], BF16, name="kt_sb")       # 0 | K^T
            v_sb = const.tile([P, NKT, H + 1], BF16, name="v_sb")  # V + ones

            nc.gpsimd.memset(kt_sb[0:64, :], 0.0)
            nc.gpsimd.memset(v_sb[:, :, H:H + 1], 1.0)

            # ---- xt DMA: 8 consumption-ordered waves on the sync ring ----
            for w in range(NW):
                nc.sync.dma_start(xt_sb[:, w, :, :], xt[:, w, :, :])

            # ---- projection work units (drip-fed between pairs) ----
            def q_proj_units(qc):
                ps = ppool.tile([P, CH], F32, tag="ps_proj")
                for c in range(CT):
                    yield lambda c=c, ps=ps: nc.tensor.matmul(
                        ps[:], lhsT=wqq_sb[:, c, :],
                        rhs=xt_qv[:, 2 * qc:2 * qc + 2, c, :, 1, :],
                        start=(c == 0), stop=(c == CT - 1))
                yield lambda ps=ps: nc.vector.tensor_copy(
                    qt_sb[:, bass.ts(qc, CH)], ps[:])

            def kv_core_units(t_i):
                ps = ppool.tile([P, CH], F32, tag="ps_proj")
                for c in range(CT):
                    yield lambda c=c, ps=ps: nc.tensor.matmul(
                        ps[:], lhsT=wkv_sb[:, c, :],
                        rhs=xt_sb[:, t_i, c, :],
                        start=(c == 0), stop=(c == CT - 1))
                # evac 1 releases the single proj PSUM slot; evac 2 derives
                # the zero-padded S^T operand from SBUF (bf16 2x DVE mode)
                yield lambda ps=ps: nc.vector.tensor_copy(
                    kvt_sb[:, bass.ts(t_i, CH)], ps[:])
                yield lambda: nc.vector.tensor_copy(
                    kt_sb[64:128, bass.ts(t_i, CH)],
                    kvt_sb[64:128, bass.ts(t_i, CH)])

            def vtile_units(t_i):
                # V^T -> V via PE transposes (full-128 contraction: the K^T
                # rows ride along and land in discarded output columns);
                # per-tile copies so each k-tile's V releases individually
                pt = tpool.tile([P, 4, H], BF16, tag="tr")
                for j in range(4):
                    yield lambda j=j, pt=pt: nc.tensor.transpose(
                        pt[:, j, :], kvt_sb[:, bass.ts(4 * t_i + j, P)],
                        ident16[:, 0:H])
                    yield lambda j=j, pt=pt: nc.vector.tensor_copy(
                        v_sb[:, 4 * t_i + j, 0:H], pt[:, j, :])

            def feeder_units(ch):
                # keep the early feed FAT: a thin ch0/ch1 feed lets the PE
                # p-state cool to 1.2GHz (measured 450ns/512-col matmuls);
                # only kv7/vt7 shift into chunk 3 to cover its exp-bound
                # stretch
                if ch == 0:
                    gens = [q_proj_units(1), kv_core_units(2),
                            vtile_units(2), kv_core_units(3)]
                elif ch == 1:
                    gens = [vtile_units(3), q_proj_units(2),
                            kv_core_units(4), vtile_units(4)]
                elif ch == 2:
                    gens = [kv_core_units(5), vtile_units(5),
                            q_proj_units(3), kv_core_units(6)]
                else:
                    gens = [vtile_units(6), kv_core_units(7),
                            vtile_units(7)]
                for g in gens:
                    yield from g

            # ---- attention: flat pipeline over k-tile PAIRS ----
            def emit_st(ch, m):
                # pair m covers k-tiles (2m, 2m+1); q-blocks below r0 are
                # fully masked and sliced out of S^T/exp/AV
                r0 = max(0, m - 4 * ch)
                ps = spool.tile([P, 2, CH], F32, name="ps_pair")
                sb = sspool.tile([P, 2, CH], BF16, tag="s_pair")
                for e in range(2):
                    nc.tensor.matmul(
                        ps[:, e, r0 * P:CH],
                        lhsT=kt_sb[:, bass.ts(2 * m + e, P)],
                        rhs=qt_sb[:, ch * CH + r0 * P:(ch + 1) * CH],
                        start=True, stop=True)
                return ps, sb, r0

            flat = [(ch, m) for ch in range(NCH) for m in range(4 * ch + 4)]
            pending = {}
            # prework: cover the first two pairs before the loop primes
            for u in kv_core_units(0):
                u()
            for u in q_proj_units(0):
                u()
            pending[flat[0]] = emit_st(*flat[0])
            for u in vtile_units(0):
                u()
            for u in kv_core_units(1):
                u()
            pending[flat[1]] = emit_st(*flat[1])
            for u in vtile_units(1):
                u()

            po = None
            osb = None
            feeder = iter(())
            per_group = 1
            for i, (ch, m) in enumerate(flat):
                n_pairs = 4 * ch + 4
                if m == 0:
                    po = opool.tile([P, 4, P], F32, name="po")
                    osb = epool.tile([P, 4, H], F32, tag="osb")
                    units = list(feeder_units(ch))
                    feeder = iter(units)
                    # cap the drip rate: chunk 0's 2-step burst used to pull
                    # kv(3) ~2.5us ahead of its xt wave, stalling the
                    # in-order PE queue on the DMA semaphore
                    per_group = max(
                        1, min(10, len(units) // max(1, n_pairs - DEPTH) + 1))
                if i + DEPTH < len(flat):
                    pending[flat[i + DEPTH]] = emit_st(*flat[i + DEPTH])
                ps, sb, r0 = pending.pop((ch, m))
                nc.scalar.activation(sb[:, :, r0 * P:CH], ps[:, :, r0 * P:CH],
                                     Exp, scale=0.125)
                diag = m >= 4 * ch
                if diag:  # diagonal block: k-tile 2m+1, q-block r0
                    blk = sb[:, 1, r0 * P:(r0 + 1) * P]
                    nc.vector.tensor_tensor(blk, blk, mask_sb[:], MULT)
                for e in range(2):
                    kt = 2 * m + e
                    for s in range(r0, 4):
                        # start=True resets the WHOLE bank's has_written
                        # bits, so only the chunk's very first AV write may
                        # set it; the other subtiles' first writes land
                        # fresh because those bits are still clear.
                        nc.tensor.matmul(
                            po[:, s, 0:H + 1],
                            lhsT=sb[:, e, bass.ts(s, P)],
                            rhs=v_sb[:, kt, :],
                            start=(kt == 0 and s == 0),
                            stop=(e == 1 and m - 4 * ch == s))
                if diag:  # q-subtile s = m-4ch is fully accumulated
                    s = m - 4 * ch
                    den = epool.tile([P, 1], F32, tag="den")
                    nc.vector.tensor_tensor(den[:], po[:, s, H:H + 1],
                                            dbias_sb[:], SUB)
                    rec = epool.tile([P, 1], F32, tag="rec")
                    nc.vector.reciprocal(rec[:], den[:])
                    nc.vector.tensor_scalar_mul(osb[:, s, :], po[:, s, 0:H],
                                                rec[:])
                    if s == 2:  # 3/4 of the chunk's output can ship early
                        nc.sync.dma_start(y_r[ch, :, 0:3, :], osb[:, 0:3, :])
                for _ in range(per_group):
                    u = next(feeder, None)
                    if u is None:
                        break
                    u()
                if m == n_pairs - 1:  # chunk done: drain feeder, store
                    for u in feeder:
                        u()
                    nc.sync.dma_start(y_r[ch, :, 3, :], osb[:, 3, :])

    nc.compile()
    return nc


def _shard_inputs(x, Wq, Wk, Wv):
    bf = ml_dtypes.bfloat16
    tri = np.tril(np.ones((P, P), dtype=np.float32)).T  # [kk,qq]=1 iff kk<=qq
    # host-side swizzles: everything per-partition-contiguous for the DMAs
    wqq = np.concatenate([Wq.T, Wq.T], axis=1).astype(bf)   # [C, 128]
    wkv = np.concatenate([Wv.T, Wk.T], axis=1).astype(bf)
    wqq = np.ascontiguousarray(wqq.reshape(CT, P, P).transpose(1, 0, 2))
    wkv = np.ascontiguousarray(wkv.reshape(CT, P, P).transpose(1, 0, 2))
    mtri = tri.astype(bf)
    in_maps = []
    for core in range(N_CORES):
        b, p = core // 2, core % 2
        if p == 0:
            # [zeros | blocks 0..30]
            xt_full = np.concatenate(
                [np.zeros((P, C), dtype=np.float32), x[b][:T - P]], axis=0).T
        else:
            xt_full = x[b].T
        xt_core = xt_full.astype(bf)  # [C, T]
        # [C, T] -> [p, wave, co, t'] with per-partition-contiguous waves
        xt_core = np.ascontiguousarray(
            xt_core.reshape(CT, P, NW, CH).transpose(1, 2, 0, 3))
        db = np.full((P, 1), 128.0 if p == 0 else 0.0, dtype=np.float32)
        in_maps.append({"xt": xt_core, "wqq": wqq, "wkv": wkv,
                        "mtri": mtri, "iden": np.eye(P, dtype=np.float32).astype(bf),
                        "dbias": db})
    return in_maps


def _unshard(results):
    y = np.zeros((B, T, H), dtype=np.float32)
    for core in range(N_CORES):
        b, p = core // 2, core % 2
        yc = results[core]["y"]
        for j in range(16):
            g = 2 * j + p
            y[b, P * g:P * g + P] = yc[P * j:P * j + P]
    return y


def kernel(x, Wq, Wk, Wv):
    global LAST_EXEC_TIME_NS, _COMPILED
    x = np.asarray(x, dtype=np.float32)
    Wq = np.asarray(Wq, dtype=np.float32)
    Wk = np.asarray(Wk, dtype=np.float32)
    Wv = np.asarray(Wv, dtype=np.float32)

    if _COMPILED is None:
        _COMPILED = _build_graph()
    nc = _COMPILED

    in_maps = _shard_inputs(x, Wq, Wk, Wv)
    kwargs = {}
    if os.environ.get("ATTN_TRACE"):
        kwargs["trace"] = True
        if os.environ.get("ATTN_TRACE_DIR"):
            kwargs["tmpdir"] = os.environ["ATTN_TRACE_DIR"]
    res = run_bass_kernel_spmd(nc, in_maps, core_ids=list(range(N_CORES)), **kwargs)
    LAST_EXEC_TIME_NS = res.exec_time_ns
    return _unshard(res.results)
